# revision 20
# baseline (speedup 1.0000x reference)
"""Trainium2 Bass kernel for the RBF-SVM dual objective (nn_Model_51556787421664).

Computes: value = 0.5 * alpha^T G alpha - sum(alpha), where
  X = xs.reshape(N, T*D) @ W            [N=4096, F=2048]
  K_ij = exp(-gamma * ||X_i - X_j||^2),  gamma = 1/F
  G = (y y^T) * K  with y in {-1,+1}

Factorization used on device (exact, no d2 materialization):
  K_ij = a_i a_j exp(2*gamma*G_ij), a_i = exp(-gamma*||X_i||^2), G = X X^T
  alpha^T G alpha = sum_ij w_i w_j exp(2*gamma*(X X^T)_ij), w_i = y_i alpha_i a_i

v2: full fp8 pipeline.
  - Encoder in fp8e4 with DoubleRowSwInterleave (host pre-interleaves W,
    pairs k-tiles; 2x PE throughput vs bf16). X is computed SCALED by 32
    (W_eff = 32*W) so the fp8 X~ tiles stay in e4m3 normal range; the RBF
    exponents use gamma' = gamma/1024 to compensate. Since sq and the gram
    both consume the SAME fp8 X~, the dominant diagonal a_i^2*exp(2g G_ii)
    = 1 stays exact.
  - Gram in fp8e4 plain DoubleRow over f-pair tiles.
  - Row sharding: each core encodes its 512 rows, AllGathers fp8 X~^T
    (two halves, overlapped with the encoder), computes its 512x4096
    gram block stripe, exp, and reduces against w on the PE.
  - All DMAs batched to ~1MB to avoid HWDGE issue-rate limits.
Host sums the 8 per-core partial vectors z and does the final dot.
"""

import functools

import numpy as np
import ml_dtypes

try:
    import jax as _jax
    if not _jax.config.jax_compilation_cache_dir:
        _jax.config.update("jax_compilation_cache_dir", "/tmp/jaxcache")
        _jax.config.update("jax_persistent_cache_min_entry_size_bytes", -1)
        _jax.config.update("jax_persistent_cache_min_compile_time_secs", 0)
except Exception:
    pass

# --- problem constants (hardcoded per contract; kernel.py is self-contained) ---
N = 4096          # rows
KDIM = 8192       # T*D contraction
F = 2048          # feature dim
NCORES = 8
NLOC = N // NCORES          # 512 local rows
P = 128
KT = KDIM // P              # 64 k-tiles
KP = KT // 2                # 32 k-pair tiles (DoubleRow)
FT = F // P                 # 16 f-tiles
FP = FT // 2                # 8 f-pair tiles
MT = NLOC // P              # 4 local row tiles
NB = N // NLOC              # 8 global row blocks
GAMMA = 1.0 / F
XSCALE = 32.0               # X~ = XSCALE * X stored in fp8
GAMMA_EFF = GAMMA / (XSCALE * XSCALE)

_FP8 = ml_dtypes.float8_e4m3


def _build_nc(reps=1, rep_a=True, rep_c=True,
              no_sq=False, w_reuse=False, gt_reuse=False, no_z=False):
    """Build the Bass module. reps>1 repeats (selected) kernel body stages
    in-NEFF for timing attribution; outputs are idempotent across reps."""
    import concourse.mybir as mybir
    import concourse.tile as tile
    from concourse import bacc

    FP8DT = mybir.dt.float8e4
    E_DT = mybir.dt.bfloat16     # exp(G) tile dtype (z-reduction operands)
    SQ_DT = mybir.dt.bfloat16    # squared-X tiles for the sq reduction
    FP32 = mybir.dt.float32
    DRSW = mybir.MatmulPerfMode.DoubleRowSwInterleave
    DR = mybir.MatmulPerfMode.DoubleRow

    nc = bacc.Bacc("TRN2", target_bir_lowering=False, debug=False,
                   num_devices=NCORES)

    # inputs (per-core): host-prepped layouts
    #  zt: [4][128, 8*1024] fp8, zt[g][p, kl*1024 + i*512 + n] =
    #       xs_flat[row n, (8g+kl)*256 + i*128 + p]  (k-pair packed Z^T)
    #  wmat: [16][128, 32*256] fp8, wmat[f][p, kp*256 + 2t + i] =
    #       32*W[kp*256 + i*128 + p, f*128 + (127-t)]  (DRSW interleaved)
    zt_d = nc.dram_tensor("zt", [4, P, 8 * 1024], FP8DT, kind="ExternalInput")
    w_d = nc.dram_tensor("wmat", [FT, P, KP * 256], FP8DT, kind="ExternalInput")
    beta_d = nc.dram_tensor("beta", [P, MT], FP32, kind="ExternalInput")
    # per-core gather indices: bofs[0, 4*d + i] = ((core + d) % 8) * 4 + i
    bofs_d = nc.dram_tensor("bofs", [1, 32], mybir.dt.int32,
                            kind="ExternalInput")
    z_out = nc.dram_tensor("z_out", [NB, NLOC], FP32, kind="ExternalOutput")
    w_out = nc.dram_tensor("w_out", [P, MT], FP32, kind="ExternalOutput")

    with tile.TileContext(nc) as tc:
        with (
            tc.tile_pool(name="persist", bufs=1) as persist,
            tc.tile_pool(name="dram", bufs=1, space="DRAM") as dram,
            tc.tile_pool(name="wstream", bufs=3) as wpool,
            tc.tile_pool(name="atmp", bufs=2) as atmp,
            tc.tile_pool(name="sqpool", bufs=1) as sqpool,
            tc.tile_pool(name="gath", bufs=3) as gath_pool,
            tc.tile_pool(name="epool", bufs=4) as epool,
            tc.tile_pool(name="ztmp", bufs=2) as ztmp,
            tc.tile_pool(name="apsum", bufs=2, space="PSUM") as apsum,
            tc.tile_pool(name="sqpsum", bufs=1, space="PSUM") as sqpsum,
            tc.tile_pool(name="gpsum", bufs=1, space="PSUM") as gpsum,
            tc.tile_pool(name="zpsum", bufs=1, space="PSUM") as zpsum,
        ):
            # ---- persistent SBUF ----
            zt_tiles = []
            for g in range(4):
                t = persist.tile([P, 8 * 1024], FP8DT, name=f"zt{g}")
                nc.sync.dma_start(t[:], zt_d[g])
                zt_tiles.append(t)
            beta_sb = persist.tile([P, MT], FP32, name="beta_sb")
            nc.sync.dma_start(beta_sb[:], beta_d[:])
            bofs_sb = persist.tile([1, 32], mybir.dt.int32, name="bofs_sb")
            nc.sync.dma_start(bofs_sb[:], bofs_d[:])
            ones_sb = persist.tile([P, 1], SQ_DT, name="ones_sb")
            nc.vector.memset(ones_sb[:], 1.0)
            # X~^T fp8, all 8 f-pair tiles in one [128, 8192] tile:
            # cols fp*1024 + i*512 + n  (pair halves adjacent)
            xpair = persist.tile([P, FP * 1024], FP8DT, name="xpair")
            w_sb = persist.tile([P, MT], E_DT, name="w_sb")
            w2_sb = persist.tile([P, MT], E_DT, name="w2_sb")

            # AG bounce buffers
            ag_state = {}

            def emit_ag_quarter(q):
                """AllGather f-pairs [2q, 2q+2) once their xpair cols are ready."""
                agin = dram.tile([2 * P, 1024], FP8DT, name=f"agin{q}")
                agout = dram.tile([NCORES * 2 * P, 1024], FP8DT,
                                  addr_space="Shared", name=f"agout{q}")
                ag_state[q] = agout
                agin_v = agin[:].rearrange("(f p) n -> f p n", f=2, p=P)
                xv = xpair[:].rearrange("p (f n) -> f p n", f=FP)
                for i in range(2):
                    nc.sync.dma_start(agin_v[i], xv[2 * q + i])
                nc.gpsimd.collective_compute(
                    "AllGather",
                    mybir.AluOpType.bypass,
                    ins=[agin[:]],
                    outs=[agout[:]],
                    replica_groups=[list(range(NCORES))],
                )

            def stage_a():
                sqx_tiles = []
                for f in range(FT):
                    wt = wpool.tile([P, KP * 256], FP8DT, tag="w", name="wt")
                    nc.sync.dma_start(wt[:], w_d[0 if w_reuse else f])
                    xp = apsum.tile([P, NLOC], FP32, tag="xp", name="xp")
                    for kp in range(KP):
                        lhsT = wt[:, kp * 256:(kp + 1) * 256].rearrange(
                            "p (i m) -> p i m", i=2)
                        g, kl = kp // 8, kp % 8
                        rhs = zt_tiles[g][:, kl * 1024:(kl + 1) * 1024].rearrange(
                            "p (i n) -> p i n", i=2)
                        nc.tensor.matmul(xp[:], lhsT, rhs,
                                         start=(kp == 0), stop=(kp == KP - 1),
                                         perf_mode=DRSW)
                    # X~^T half-tile, rounded to fp8 (consistent with gram)
                    xcol = xpair[:, f * 512:(f + 1) * 512]
                    nc.scalar.activation(xcol, xp[:],
                                         mybir.ActivationFunctionType.Copy)
                    if not no_sq:
                        t = sqpool.tile([P, NLOC], SQ_DT, tag=f"sqx{f}",
                                        name=f"sqx{f}")
                        sqx_tiles.append(t)
                        nc.scalar.activation(
                            t[:], xcol,
                            mybir.ActivationFunctionType.Square,
                            scale=1.0 / XSCALE)
                    if f % 4 == 3:
                        emit_ag_quarter(f // 4)

                # sq reduction: bf16 ones-matmuls, one m-group at a time;
                # each group's result is copied out of PSUM before the next
                # group's start=True can zero any part of the bank.
                sq_sb = atmp.tile([P, MT], FP32, tag="sqs", name="sq_sb")
                if not no_sq:
                    for m in range(MT):
                        sp = sqpsum.tile([P, 1], FP32, tag="sp", name="sp")
                        for f in range(FT):
                            nc.tensor.matmul(
                                sp[:, 0:1],
                                sqx_tiles[f][:, m * P:(m + 1) * P],
                                ones_sb[:, 0:1],
                                start=(f == 0), stop=(f == FT - 1))
                        nc.vector.tensor_copy(sq_sb[:, m:m + 1], sp[:, 0:1])

                # w = beta * exp(-gamma*sq8)   (sq8 = sum (x~/32)^2 = sq)
                a_sb = atmp.tile([P, MT], FP32, tag="a", name="a_sb")
                nc.scalar.activation(a_sb[:], sq_sb[:],
                                     mybir.ActivationFunctionType.Exp,
                                     scale=-GAMMA)
                nc.vector.tensor_mul(w_sb[:], a_sb[:], beta_sb[:])
                nc.vector.tensor_add(w2_sb[:], w_sb[:], w_sb[:])
                wf32 = atmp.tile([P, MT], FP32, tag="wf32", name="wf32")
                nc.vector.tensor_copy(wf32[:], w_sb[:])
                nc.sync.dma_start(w_out[:], wf32[:])

            rep_ctr = [0]

            def stage_c():
                import concourse.bass as bass
                rc_id = rep_ctr[0]
                rep_ctr[0] += 1
                for d in range(5):
                    # gather block (core+d)%8 via register-indexed DMA
                    gt = gath_pool.tile([P, FP * 1024], FP8DT, tag="gt",
                                        name="gt")
                    with nc.sync.register(f"gtoff{rc_id}_{d}") as off_reg:
                        for q in range(4):
                            agflat = ag_state[q][:].rearrange(
                                "(bf p) n -> bf p n", p=P)
                            for i in range(2):
                                f = 2 * q + i
                                nc.sync.reg_load(
                                    off_reg,
                                    bofs_sb[0:1, 4 * d + i:4 * d + i + 1])
                                off = nc.sync.snap(off_reg)
                                nc.sync.dma_start(
                                    gt[:, f * 1024:(f + 1) * 1024],
                                    agflat[bass.ds(off, 1)])
                    gps = [gpsum.tile([P, NLOC], FP32, tag=f"g{m}", name=f"g{m}")
                           for m in range(MT)]
                    for fp in range(FP):
                        rhs = gt[:, fp * 1024:(fp + 1) * 1024].rearrange(
                            "p (i n) -> p i n", i=2)
                        for m in range(MT):
                            lhsT = xpair[:].rearrange(
                                "p (f i n) -> p f i n", f=FP, i=2)[
                                :, fp, :, m * P:(m + 1) * P]
                            nc.tensor.matmul(
                                gps[m][:], lhsT, rhs,
                                start=(fp == 0), stop=(fp == FP - 1),
                                perf_mode=DR)
                    wloc = w2_sb if d in (1, 2, 3) else w_sb
                    zp = zpsum.tile([1, NLOC], FP32, tag="z", name="zp")
                    for m in range(MT):
                        et = epool.tile([P, NLOC], E_DT, tag=f"e{m}",
                                        name=f"et{m}")
                        nc.scalar.activation(
                            et[:], gps[m][:],
                            mybir.ActivationFunctionType.Exp,
                            scale=2.0 * GAMMA_EFF)
                        if not no_z:
                            nc.tensor.matmul(zp[:], wloc[:, m:m + 1], et[:],
                                             start=(m == 0), stop=(m == MT - 1))
                    if not no_z:
                        zs = ztmp.tile([1, NLOC], FP32, tag="zs", name="zs")
                        nc.vector.tensor_copy(zs[:], zp[:])
                        nc.sync.dma_start(z_out[d:d + 1, :], zs[:])

            for rep in range(reps):
                if rep_a or rep == 0:
                    stage_a()   # includes the two AllGather halves
                if rep_c or rep == 0:
                    stage_c()

    nc.compile()
    return nc


@functools.cache
def _get_nc():
    return _build_nc()


def _prep_in_maps(xs, W, ys, alphas):
    xs = np.asarray(xs, dtype=np.float32)
    W = np.asarray(W, dtype=np.float32)
    ys = np.asarray(ys)
    alphas = np.asarray(alphas, dtype=np.float32)

    beta = ((2 * ys - 1).astype(np.float32) * alphas)  # [N]

    # W_eff = 32*W, DRSW interleaved: wmat[f][p, kp*256 + 2t + i] =
    #   W_eff[kp*256 + i*128 + p, f*128 + (127-t)]
    w_eff = (W * XSCALE).astype(_FP8)                  # [8192, 2048]
    wv = w_eff.reshape(KP, 2, P, FT, P)                # [kp, i, p, f, m]
    wv = wv[:, :, :, :, ::-1]                          # reverse m -> t
    # target [f][p][kp][t][i]
    w_t = np.ascontiguousarray(wv.transpose(3, 2, 0, 4, 1)).reshape(
        FT, P, KP * 256)

    # Z^T k-pair packed: zt[g][p, kl*1024 + i*512 + n] =
    #   xs_flat[row n, (8g+kl)*256 + i*128 + p]
    zf = xs.reshape(N, KDIM).astype(_FP8)              # [n, k]
    zv = zf.T.reshape(4, 8, 2, P, N)                   # [g, kl, i, p, n]

    in_maps = []
    for c in range(NCORES):
        sl = slice(c * NLOC, (c + 1) * NLOC)
        zt_c = np.ascontiguousarray(
            zv[:, :, :, :, sl].transpose(0, 3, 1, 2, 4)).reshape(4, P, 8 * 1024)
        beta_c = np.ascontiguousarray(beta[sl].reshape(MT, P).T)  # [P, MT]
        bofs_c = np.zeros((1, 32), np.int32)
        for d in range(5):
            for i in range(2):
                bofs_c[0, 4 * d + i] = ((c + d) % NCORES) * 2 + i
        in_maps.append({"zt": zt_c, "wmat": w_t, "beta": beta_c,
                        "bofs": bofs_c})
    return in_maps, alphas


def _finish(results, alphas):
    z_total = np.zeros(N, dtype=np.float64)
    w_full = np.zeros(N, dtype=np.float64)
    for c, r in enumerate(results):
        zo = r["z_out"].astype(np.float64)
        for d in range(5):
            b = (c + d) % NCORES
            z_total[b * NLOC:(b + 1) * NLOC] += zo[d]
        sl = slice(c * NLOC, (c + 1) * NLOC)
        w_full[sl] = r["w_out"].astype(np.float64).T.reshape(NLOC)
    value = 0.5 * float(np.dot(w_full, z_total)) - float(
        np.sum(alphas.astype(np.float64)))
    return np.array([[value]], dtype=np.float32)


class Runner:
    """Compiles once; keeps inputs on device for repeated timed execs."""

    def __init__(self):
        self.nc = _get_nc()
        self._jitted = None

    def run_spmd(self, in_maps):
        from concourse import bass_utils
        res = bass_utils.run_bass_kernel_spmd(
            self.nc, in_maps, core_ids=list(range(NCORES)))
        return res.results

    # -- custom PJRT path mirroring bass2jax.run_bass_via_pjrt, but keeping
    #    the jitted fn + device inputs so repeated executions can be timed --
    def prepare(self, in_maps):
        import jax
        import numpy as np
        from jax.sharding import Mesh, PartitionSpec
        from jax.experimental.shard_map import shard_map
        import concourse.mybir as mybir
        from concourse import bass2jax

        nc = self.nc
        bass2jax.install_neuronx_cc_hook()
        partition_name = (nc.partition_id_tensor.name
                          if nc.partition_id_tensor else None)
        in_names, out_names, out_avals, zero_outs = [], [], [], []
        for alloc in nc.m.functions[0].allocations:
            if not isinstance(alloc, mybir.MemoryLocationSet):
                continue
            name = alloc.memorylocations[0].name
            if alloc.kind == "ExternalInput":
                if name != partition_name:
                    in_names.append(name)
            elif alloc.kind == "ExternalOutput":
                out_names.append(name)
                shape = tuple(alloc.tensor_shape)
                dtype = mybir.dt.np(alloc.dtype)
                out_avals.append(jax.core.ShapedArray(shape, dtype))
                zero_outs.append(np.zeros(shape, dtype))
        n_params = len(in_names)
        n_outs = len(out_avals)
        all_names = in_names + out_names
        if partition_name is not None:
            all_names = all_names + [partition_name]

        def _body(*args):
            operands = list(args)
            if partition_name is not None:
                operands.append(bass2jax.partition_id_tensor())
            outs = bass2jax._bass_exec_p.bind(
                *operands,
                out_avals=tuple(out_avals),
                in_names=tuple(all_names),
                out_names=tuple(out_names),
                lowering_input_output_aliases=(),
                sim_require_finite=True,
                sim_require_nnan=True,
                nc=nc,
            )
            return tuple(outs)

        devices = jax.devices()[:NCORES]
        mesh = Mesh(np.asarray(devices), ("core",))
        in_specs = (PartitionSpec("core"),) * (n_params + n_outs)
        out_specs = (PartitionSpec("core"),) * n_outs
        donate = tuple(range(n_params, n_params + n_outs))
        self._jitted = jax.jit(
            shard_map(_body, mesh=mesh, in_specs=in_specs,
                      out_specs=out_specs, check_rep=False),
            donate_argnums=donate, keep_unused=True)
        concat_in = [
            np.concatenate([np.asarray(in_maps[c][nm]) for c in range(NCORES)],
                           axis=0)
            for nm in in_names
        ]
        self._sharding = jax.sharding.NamedSharding(mesh, PartitionSpec("core"))
        self._dev_in = [jax.device_put(a, self._sharding) for a in concat_in]
        self._zero_outs = zero_outs
        self._out_names = out_names
        self._out_avals = out_avals

    def _zeros_dev(self):
        import jax
        return [jax.device_put(
                    np.zeros((NCORES * z.shape[0], *z.shape[1:]), z.dtype),
                    self._sharding)
                for z in self._zero_outs]

    def exec_once(self):
        out_arrs = self._jitted(*self._dev_in, *self._zeros_dev())
        import jax
        jax.block_until_ready(out_arrs)
        return [
            {nm: np.asarray(out_arrs[i]).reshape(NCORES, *self._out_avals[i].shape)[c]
             for i, nm in enumerate(self._out_names)}
            for c in range(NCORES)
        ]

    def time(self, reps=10):
        import time
        self.exec_once()  # warm
        ts = []
        for _ in range(reps):
            zo = self._zeros_dev()
            import jax
            jax.block_until_ready(zo)
            t0 = time.perf_counter()
            out = self._jitted(*self._dev_in, *zo)
            jax.block_until_ready(out)
            ts.append(time.perf_counter() - t0)
        return min(ts), sorted(ts)[len(ts) // 2]


def kernel(**inputs) -> np.ndarray:
    in_maps, alphas = _prep_in_maps(**inputs)
    r = Runner()
    results = r.run_spmd(in_maps)
    return _finish(results, alphas)


if __name__ == "__main__":
    rng = np.random.default_rng(0)
    xs = rng.standard_normal((N, 64, 128), dtype=np.float32)
    W = (rng.standard_normal((KDIM, F), dtype=np.float32) / np.sqrt(KDIM)).astype(np.float32)
    ys = rng.integers(0, 2, N).astype(np.int32)
    alphas = rng.standard_normal(N, dtype=np.float32)
    out = kernel(xs=xs, W=W, ys=ys, alphas=alphas)
    print("kernel out:", out)
